# revision 3
# baseline (speedup 1.0000x reference)
"""Builder for the sparse-attention SPMD kernel (one NeuronCore program).

Per core: B batches x NH heads, processed as head-pairs (A, B):
  - QK projections fp32, col-packed across the head pair, interleaved groups
  - S = Q^T.T @ K^T fp32 K=64, row-packed A/B adjacent (PSUM rows via SBUF partition halves)
  - mask add fused into the S PSUM group via identity-matmul accumulate (bf16, K=128)
  - DVE: negmax = -max(S_masked) (tensor_reduce negate), from PSUM
  - ACT: P = exp(S_masked + negmax) straight from PSUM, accum(sum) -> rowsum
  - GpSimd: Pn = P * (1/rowsum)   (per-partition scalar, bf16)
  - transpose Pn -> PT per q-tile (DMA xbar, alternating sync/scalar rings)
  - AV: O^T[d, q] = sum_m V[m, d] * PT[m, q], bf16 K=128, col-packed interleaved
Output written as O^T [B, NH, 64, G]; host transposes to [B, NH, G, 64].
"""
import sys

sys.path.insert(0, '/opt/trn_rl_repo')
from contextlib import ExitStack

import concourse.bass as bass
import concourse.tile as tile
from concourse import bacc, mybir

FP32 = mybir.dt.float32
BF16 = mybir.dt.bfloat16
I32 = mybir.dt.int32
AF = mybir.ActivationFunctionType
ALU = mybir.AluOpType


def build_attention(B=2, NH=8, G=1024, I=256, D=64,
                    pss_bufs=3, p_bufs=3, pt_bufs=2, qk_bufs=2, pn_bufs=3):
    """DRAM params: hT [B,I,G] f32, mask [G,G] i32, wq/wk/wv [NH,I,D] f32
    (wq pre-scaled by 1/sqrt(D)), ident [128,128] bf16; out [B,NH,D,G] f32 (O^T)."""
    assert D == 64 and I % 128 == 0 and G % 512 == 0 and NH % 2 == 0
    KT = I // 128          # contraction k-tiles for projections
    QT = G // 128          # q tiles
    MC = G // 512          # m chunks of 512
    QC = G // 512          # q chunks of 512 (for AV)
    MT = G // 128          # m tiles
    assert MT <= 8

    nc = bacc.Bacc(None, target_bir_lowering=False, debug=False)
    hT_ext = nc.declare_dram_parameter("hT", [B, I, G], FP32, isOutput=False)
    mask_ext = nc.declare_dram_parameter("mask", [G, G], I32, isOutput=False)
    wq_ext = nc.declare_dram_parameter("wq", [NH, I, D], FP32, isOutput=False)
    wk_ext = nc.declare_dram_parameter("wk", [NH, I, D], FP32, isOutput=False)
    wv_ext = nc.declare_dram_parameter("wv", [NH, I, D], FP32, isOutput=False)
    id_ext = nc.declare_dram_parameter("ident", [128, 128], BF16, isOutput=False)
    out_ext = nc.declare_dram_parameter("out", [B, NH, D, G], FP32, isOutput=True)

    ctx = ExitStack()
    with ctx:
        tc = ctx.enter_context(tile.TileContext(nc))
        const = ctx.enter_context(tc.tile_pool(name="const", bufs=1))
        stage = ctx.enter_context(tc.tile_pool(name="stage", bufs=2))
        qk_pool = ctx.enter_context(tc.tile_pool(name="qk", bufs=qk_bufs))
        p_pool = ctx.enter_context(tc.tile_pool(name="p", bufs=p_bufs))
        pn_pool = ctx.enter_context(tc.tile_pool(name="pn", bufs=pn_bufs))
        pt_pool = ctx.enter_context(tc.tile_pool(name="pt", bufs=pt_bufs))
        v_pool = ctx.enter_context(tc.tile_pool(name="v", bufs=2))
        o_pool = ctx.enter_context(tc.tile_pool(name="o", bufs=2))
        st_pool = ctx.enter_context(tc.tile_pool(name="stats", bufs=2))
        # PSUM budget (8 banks): pss 3x2 + proj 1 + v/o shared 1 = 8
        ps_proj = ctx.enter_context(tc.tile_pool(name="psproj", bufs=1, space="PSUM"))
        ps_s = ctx.enter_context(tc.tile_pool(name="pss", bufs=pss_bufs, space="PSUM"))
        ps_vo = ctx.enter_context(tc.tile_pool(name="psvo", bufs=1, space="PSUM"))

        # ---------- setup: load inputs (gpsimd SWDGE ring) ----------
        hT_sb = const.tile([128, B, KT, G], FP32)
        for b in range(B):
            nc.gpsimd.dma_start(
                out=hT_sb[:, b],
                in_=hT_ext[b].rearrange("(kt p) g -> p kt g", p=128),
            )
        hTb_sb = const.tile([128, B, KT, G], BF16)
        for b in range(B):
            nc.vector.tensor_copy(hTb_sb[:, b], hT_sb[:, b])

        wq_sb = const.tile([128, NH, KT, D], FP32)
        wk_sb = const.tile([128, NH, KT, D], FP32)
        wv_f32 = stage.tile([128, NH, KT, D], FP32)
        wv_sb = const.tile([128, NH, KT, D], BF16)
        nc.gpsimd.dma_start(out=wq_sb[:], in_=wq_ext.rearrange("h (kt p) d -> p h kt d", p=128))
        nc.gpsimd.dma_start(out=wk_sb[:], in_=wk_ext.rearrange("h (kt p) d -> p h kt d", p=128))
        nc.gpsimd.dma_start(out=wv_f32[:], in_=wv_ext.rearrange("h (kt p) d -> p h kt d", p=128))
        nc.vector.tensor_copy(wv_sb[:], wv_f32[:])

        ident_sb = const.tile([128, 128], BF16)
        nc.gpsimd.dma_start(out=ident_sb[:], in_=id_ext[:])

        # mask -> maskbias (bf16): -1e30 where masked, 0 where allowed
        mbp_sb = const.tile([128, QT, G], BF16)
        for qt in range(QT):
            m_i32 = stage.tile([128, G], I32, tag="mstage")
            nc.gpsimd.dma_start(out=m_i32[:], in_=mask_ext[qt * 128:(qt + 1) * 128, :])
            nc.scalar.activation(mbp_sb[:, qt], m_i32[:], AF.Copy, bias=0.0, scale=-1.0e30)

        tp_engines = [nc.sync, nc.scalar]
        tp_i = 0

        # ---------- main loop over (batch, head-pair) ----------
        for b in range(B):
            for hp in range(NH // 2):
                hA, hB = 2 * hp, 2 * hp + 1
                # --- QK projections (fp32, col-packed A/B interleaved) ---
                qk_sb = qk_pool.tile([128, 2 * G], FP32, tag="qk")
                for W, w_sb, off in (("q", wq_sb, 0), ("k", wk_sb, G)):
                    for ch in range(G // 512):
                        sl = slice(512 * ch, 512 * (ch + 1))
                        psp = ps_proj.tile([128, 512], FP32, tag="proj", name=f"psp{W}{ch}")
                        for kt in range(KT):
                            st, sp = kt == 0, kt == KT - 1
                            nc.tensor.matmul(psp[0:64, :], w_sb[:, hA, kt],
                                             hT_sb[:, b, kt, sl], start=st, stop=sp,
                                             skip_group_check=True)
                            nc.tensor.matmul(psp[64:128, :], w_sb[:, hB, kt],
                                             hT_sb[:, b, kt, sl], start=st, stop=sp,
                                             skip_group_check=True)
                        # rows 0:64 = pair A (Q^T | K^T), rows 64:128 = pair B
                        nc.vector.tensor_copy(qk_sb[:, off + 512 * ch:off + 512 * (ch + 1)], psp[:])

                # --- V projections (bf16) ---
                v_sb = {}
                for X, hX in (("A", hA), ("B", hB)):
                    psv = ps_vo.tile([128, MT * 64], FP32, tag="vo", name=f"psv{X}")
                    for mt in range(MT):
                        for kt in range(KT):
                            nc.tensor.matmul(
                                psv[:, 64 * mt:64 * (mt + 1)],
                                hTb_sb[:, b, kt, 128 * mt:128 * (mt + 1)],
                                wv_sb[:, hX, kt],
                                start=(kt == 0), stop=(kt == KT - 1),
                            )
                    vt = v_pool.tile([128, MT * 64], BF16, tag="v", name=f"v{X}")
                    nc.scalar.copy(vt[:], psv[:])
                    v_sb[X] = vt

                # --- S + mask + softmax per q-tile; A/B row-packed adjacent ---
                negmax = {X: st_pool.tile([128, QT], FP32, tag="negmax", name=f"negmax{X}") for X in "AB"}
                rowsum = {X: st_pool.tile([128, QT], FP32, tag="rowsum", name=f"rowsum{X}") for X in "AB"}
                P = {X: p_pool.tile([128, QT, G], BF16, tag="p", name=f"P{X}") for X in "AB"}
                rows_of = {"A": slice(0, 64), "B": slice(64, 128)}
                for qt in range(QT):
                    pss = {X: ps_s.tile([128, G], FP32, tag="s", name=f"pss{X}") for X in "AB"}
                    for ch in range(MC):
                        sl = slice(512 * ch, 512 * (ch + 1))
                        for X in "AB":
                            rows = rows_of[X]
                            nc.tensor.matmul(
                                pss[X][:, sl],
                                qk_sb[rows, 128 * qt:128 * (qt + 1)],
                                qk_sb[rows, G + 512 * ch: G + 512 * (ch + 1)],
                                start=True, stop=False, skip_group_check=True,
                            )
                    for ch in range(MC):
                        sl = slice(512 * ch, 512 * (ch + 1))
                        for X in "AB":
                            nc.tensor.matmul(
                                pss[X][:, sl], ident_sb[:],
                                mbp_sb[:, qt, sl],
                                start=False, stop=True, skip_group_check=True,
                            )
                    for X in "AB":
                        nc.vector.tensor_reduce(
                            negmax[X][:, qt:qt + 1], pss[X][:],
                            axis=mybir.AxisListType.X, op=ALU.max, negate=True,
                        )
                        nc.scalar.activation(
                            P[X][:, qt], pss[X][:], AF.Exp,
                            bias=negmax[X][:, qt:qt + 1], scale=1.0,
                            accum_out=rowsum[X][:, qt:qt + 1],
                        )

                # --- recip + normalize (gpsimd) + transpose (alternating rings) ---
                rcp = {}
                for X in "AB":
                    r = st_pool.tile([128, QT], FP32, tag="rcp", name=f"rcp{X}")
                    nc.vector.reciprocal(r[:], rowsum[X][:])
                    rcp[X] = r
                # PT layout: [m_in, mt, qc, q_in(512)] so AV rhs slices are contiguous
                PT = {X: pt_pool.tile([128, MT, QC, 512], BF16, tag="pt", name=f"PT{X}") for X in "AB"}
                for X in "AB":
                    for qt in range(QT):
                        pn = pn_pool.tile([128, G], BF16, tag="pn")
                        nc.gpsimd.tensor_scalar_mul(pn[:], P[X][:, qt], rcp[X][:, qt:qt + 1])
                        qc, qi = qt // 4, (qt % 4) * 128
                        tp_engines[tp_i % 2].dma_start_transpose(
                            out=PT[X][:, :, qc, qi:qi + 128], in_=pn[:])
                        tp_i += 1

                # --- AV (bf16 K=128, col-packed interleaved) + output ---
                o_sb = o_pool.tile([128, QC, 512], FP32, tag="o")
                for qc in range(QC):
                    pso = ps_vo.tile([128, 512], FP32, tag="vo", name=f"pso{qc}")
                    for mkt in range(MT):
                        st, sp = mkt == 0, mkt == MT - 1
                        nc.tensor.matmul(
                            pso[0:64, :], v_sb["A"][:, 64 * mkt:64 * (mkt + 1)],
                            PT["A"][:, mkt, qc, :],
                            start=st, stop=sp, skip_group_check=True,
                        )
                        nc.tensor.matmul(
                            pso[64:128, :], v_sb["B"][:, 64 * mkt:64 * (mkt + 1)],
                            PT["B"][:, mkt, qc, :],
                            start=st, stop=sp, skip_group_check=True,
                        )
                    nc.scalar.copy(o_sb[:, qc], pso[:])
                nc.gpsimd.dma_start(
                    out=out_ext[b, hA].rearrange("d (qc qi) -> d qc qi", qc=QC),
                    in_=o_sb[0:64],
                )
                nc.gpsimd.dma_start(
                    out=out_ext[b, hB].rearrange("d (qc qi) -> d qc qi", qc=QC),
                    in_=o_sb[64:128],
                )

    nc.compile()
    return nc


# ---------------------------------------------------------------------------
# Host-side wrapper: shard over batch across 8 cores, run SPMD, gather.
# ---------------------------------------------------------------------------
import numpy as np
import ml_dtypes

N_CORES = 8
_B_FULL, _NH, _G, _I, _D = 16, 8, 1024, 256, 64
_B_PER_CORE = _B_FULL // N_CORES

_cached_nc = None


def _get_nc():
    global _cached_nc
    if _cached_nc is None:
        _cached_nc = build_attention(B=_B_PER_CORE, NH=_NH, G=_G, I=_I, D=_D)
    return _cached_nc


def _make_in_maps(h, mask, W_Q, W_K, W_V):
    hT = np.ascontiguousarray(np.transpose(np.asarray(h, np.float32), (0, 2, 1)))
    wq = np.ascontiguousarray(np.asarray(W_Q, np.float32) / np.sqrt(np.float32(_D)))
    wk = np.ascontiguousarray(np.asarray(W_K, np.float32))
    wv = np.ascontiguousarray(np.asarray(W_V, np.float32))
    mask_i = np.ascontiguousarray(np.asarray(mask, np.int32))
    ident = np.eye(128).astype(ml_dtypes.bfloat16)
    return [
        {
            "hT": np.ascontiguousarray(hT[c * _B_PER_CORE:(c + 1) * _B_PER_CORE]),
            "mask": mask_i,
            "wq": wq,
            "wk": wk,
            "wv": wv,
            "ident": ident,
        }
        for c in range(N_CORES)
    ]


def kernel(h, mask, W_Q, W_K, W_V):
    """h [16,1024,256] f32, mask [1024,1024] i32, W_* [8,256,64] f32
    -> [16, 8, 1024, 64] f32"""
    from concourse.bass_utils import run_bass_kernel_spmd

    nc = _get_nc()
    in_maps = _make_in_maps(h, mask, W_Q, W_K, W_V)
    res = run_bass_kernel_spmd(nc, in_maps, core_ids=list(range(N_CORES)))
    outs = [np.asarray(res.results[c]["out"]).reshape(_B_PER_CORE, _NH, _D, _G)
            for c in range(N_CORES)]
    full = np.concatenate(outs, axis=0)              # [16, NH, D, G]
    return np.ascontiguousarray(full.transpose(0, 1, 3, 2)).astype(np.float32)


# revision 6
# speedup vs baseline: 4.1400x; 4.1400x over previous
"""Builder for the sparse-attention SPMD kernel (one NeuronCore program).

Per core: B batches x NH heads, processed as head-pairs (A, B):
  - QK projections fp32, col-packed across the head pair, interleaved groups
  - S = Q^T.T @ K^T fp32 K=64, row-packed A/B adjacent (PSUM rows via SBUF partition halves)
  - mask add fused into the S PSUM group via identity-matmul accumulate (bf16, K=128)
  - DVE: negmax = -max(S_masked) (tensor_reduce negate), from PSUM
  - ACT: P = exp(S_masked + negmax) straight from PSUM, accum(sum) -> rowsum
  - DVE: Pn = P * (1/rowsum)   (per-partition scalar, bf16)
  - transpose Pn -> PT per q-tile (DMA xbar, alternating sync/scalar rings)
  - AV: O^T[d, q] = sum_m V[m, d] * PT[m, q], bf16 K=128, col-packed interleaved
Output written as O^T [B, NH, 64, G]; host transposes to [B, NH, G, 64].
"""
import sys

sys.path.insert(0, '/opt/trn_rl_repo')
from contextlib import ExitStack

import concourse.bass as bass
import concourse.tile as tile
from concourse import bacc, mybir

FP32 = mybir.dt.float32
BF16 = mybir.dt.bfloat16
I32 = mybir.dt.int32
AF = mybir.ActivationFunctionType
ALU = mybir.AluOpType


def build_attention(B=2, NH=8, G=1024, I=256, D=64,
                    pss_bufs=3, p_bufs=3, pt_bufs=2, qk_bufs=2, pn_bufs=3):
    """DRAM params: hT [B,I,G] f32, mask [G,G] i32, wq/wk/wv [NH,I,D] f32
    (wq pre-scaled by 1/sqrt(D)), ident [128,128] bf16; out [B,NH,D,G] f32 (O^T)."""
    assert D == 64 and I % 128 == 0 and G % 512 == 0 and NH % 2 == 0
    KT = I // 128          # contraction k-tiles for projections
    QT = G // 128          # q tiles
    MC = G // 512          # m chunks of 512
    QC = G // 512          # q chunks of 512 (for AV)
    MT = G // 128          # m tiles
    assert MT <= 8

    nc = bacc.Bacc(None, target_bir_lowering=False, debug=False)
    hT_ext = nc.declare_dram_parameter("hT", [B, I, G], FP32, isOutput=False)
    mask_ext = nc.declare_dram_parameter("mask", [G, G], I32, isOutput=False)
    wq_ext = nc.declare_dram_parameter("wq", [NH, I, D], FP32, isOutput=False)
    wk_ext = nc.declare_dram_parameter("wk", [NH, I, D], FP32, isOutput=False)
    wv_ext = nc.declare_dram_parameter("wv", [NH, I, D], FP32, isOutput=False)
    id_ext = nc.declare_dram_parameter("ident", [128, 128], BF16, isOutput=False)
    out_ext = nc.declare_dram_parameter("out", [B, NH, D, G], FP32, isOutput=True)

    ctx = ExitStack()
    with ctx:
        tc = ctx.enter_context(tile.TileContext(nc))
        const = ctx.enter_context(tc.tile_pool(name="const", bufs=1))
        stage = ctx.enter_context(tc.tile_pool(name="stage", bufs=2))
        qk_pool = ctx.enter_context(tc.tile_pool(name="qk", bufs=qk_bufs))
        p_pool = ctx.enter_context(tc.tile_pool(name="p", bufs=p_bufs))
        pn_pool = ctx.enter_context(tc.tile_pool(name="pn", bufs=pn_bufs))
        pt_pool = ctx.enter_context(tc.tile_pool(name="pt", bufs=pt_bufs))
        v_pool = ctx.enter_context(tc.tile_pool(name="v", bufs=2))
        o_pool = ctx.enter_context(tc.tile_pool(name="o", bufs=2))
        st_pool = ctx.enter_context(tc.tile_pool(name="stats", bufs=2))
        # PSUM budget (8 banks): pss 3x2 + proj 1 + v/o shared 1 = 8
        ps_proj = ctx.enter_context(tc.tile_pool(name="psproj", bufs=1, space="PSUM"))
        ps_s = ctx.enter_context(tc.tile_pool(name="pss", bufs=pss_bufs, space="PSUM"))
        ps_vo = ctx.enter_context(tc.tile_pool(name="psvo", bufs=1, space="PSUM"))

        # ---------- setup: load inputs (gpsimd SWDGE ring) ----------
        hT_sb = const.tile([128, B, KT, G], FP32)
        for b in range(B):
            nc.gpsimd.dma_start(
                out=hT_sb[:, b],
                in_=hT_ext[b].rearrange("(kt p) g -> p kt g", p=128),
            )
        hTb_sb = const.tile([128, B, KT, G], BF16)
        for b in range(B):
            nc.vector.tensor_copy(hTb_sb[:, b], hT_sb[:, b])

        wq_sb = const.tile([128, NH, KT, D], FP32)
        wk_sb = const.tile([128, NH, KT, D], FP32)
        wv_f32 = stage.tile([128, NH, KT, D], FP32)
        wv_sb = const.tile([128, NH, KT, D], BF16)
        nc.gpsimd.dma_start(out=wq_sb[:], in_=wq_ext.rearrange("h (kt p) d -> p h kt d", p=128))
        nc.gpsimd.dma_start(out=wk_sb[:], in_=wk_ext.rearrange("h (kt p) d -> p h kt d", p=128))
        nc.gpsimd.dma_start(out=wv_f32[:], in_=wv_ext.rearrange("h (kt p) d -> p h kt d", p=128))
        nc.vector.tensor_copy(wv_sb[:], wv_f32[:])

        ident_sb = const.tile([128, 128], BF16)
        nc.gpsimd.dma_start(out=ident_sb[:], in_=id_ext[:])

        # mask -> maskbias (bf16): -1e30 where masked, 0 where allowed
        mbp_sb = const.tile([128, QT, G], BF16)
        for qt in range(QT):
            m_i32 = stage.tile([128, G], I32, tag="mstage")
            nc.gpsimd.dma_start(out=m_i32[:], in_=mask_ext[qt * 128:(qt + 1) * 128, :])
            nc.scalar.activation(mbp_sb[:, qt], m_i32[:], AF.Copy, bias=0.0, scale=-1.0e30)

        tp_engines = [nc.sync]
        tp_i = 0

        # ---------- main loop over (batch, head-pair) ----------
        for b in range(B):
            for hp in range(NH // 2):
                hA, hB = 2 * hp, 2 * hp + 1
                # --- QK projections (fp32, col-packed A/B interleaved) ---
                qk_sb = qk_pool.tile([128, 2 * G], FP32, tag="qk")
                for W, w_sb, off in (("q", wq_sb, 0), ("k", wk_sb, G)):
                    for ch in range(G // 512):
                        sl = slice(512 * ch, 512 * (ch + 1))
                        psp = ps_proj.tile([128, 512], FP32, tag="proj", name=f"psp{W}{ch}")
                        for kt in range(KT):
                            st, sp = kt == 0, kt == KT - 1
                            nc.tensor.matmul(psp[0:64, :], w_sb[:, hA, kt],
                                             hT_sb[:, b, kt, sl], start=st, stop=sp,
                                             skip_group_check=True)
                            nc.tensor.matmul(psp[64:128, :], w_sb[:, hB, kt],
                                             hT_sb[:, b, kt, sl], start=st, stop=sp,
                                             skip_group_check=True)
                        # rows 0:64 = pair A (Q^T | K^T), rows 64:128 = pair B
                        nc.vector.tensor_copy(qk_sb[:, off + 512 * ch:off + 512 * (ch + 1)], psp[:])

                # --- V projections (bf16) ---
                v_sb = {}
                for X, hX in (("A", hA), ("B", hB)):
                    psv = ps_vo.tile([128, MT * 64], FP32, tag="vo", name=f"psv{X}")
                    for mt in range(MT):
                        for kt in range(KT):
                            nc.tensor.matmul(
                                psv[:, 64 * mt:64 * (mt + 1)],
                                hTb_sb[:, b, kt, 128 * mt:128 * (mt + 1)],
                                wv_sb[:, hX, kt],
                                start=(kt == 0), stop=(kt == KT - 1),
                            )
                    vt = v_pool.tile([128, MT * 64], BF16, tag="v", name=f"v{X}")
                    nc.scalar.copy(vt[:], psv[:])
                    v_sb[X] = vt

                # --- S + mask + softmax per q-tile; A/B row-packed adjacent ---
                negmax = {X: st_pool.tile([128, QT], FP32, tag="negmax", name=f"negmax{X}") for X in "AB"}
                rowsum = {X: st_pool.tile([128, QT], FP32, tag="rowsum", name=f"rowsum{X}") for X in "AB"}
                P = {X: p_pool.tile([128, QT, G], BF16, tag="p", name=f"P{X}") for X in "AB"}
                rows_of = {"A": slice(0, 64), "B": slice(64, 128)}
                for qt in range(QT):
                    pss = {X: ps_s.tile([128, G], FP32, tag="s", name=f"pss{X}") for X in "AB"}
                    for ch in range(MC):
                        sl = slice(512 * ch, 512 * (ch + 1))
                        for X in "AB":
                            rows = rows_of[X]
                            nc.tensor.matmul(
                                pss[X][:, sl],
                                qk_sb[rows, 128 * qt:128 * (qt + 1)],
                                qk_sb[rows, G + 512 * ch: G + 512 * (ch + 1)],
                                start=True, stop=False, skip_group_check=True,
                            )
                    for ch in range(MC):
                        sl = slice(512 * ch, 512 * (ch + 1))
                        for X in "AB":
                            nc.tensor.matmul(
                                pss[X][:, sl], ident_sb[:],
                                mbp_sb[:, qt, sl],
                                start=False, stop=True, skip_group_check=True,
                            )
                    for X in "AB":
                        nc.vector.tensor_reduce(
                            negmax[X][:, qt:qt + 1], pss[X][:],
                            axis=mybir.AxisListType.X, op=ALU.max, negate=True,
                        )
                        nc.scalar.activation(
                            P[X][:, qt], pss[X][:], AF.Exp,
                            bias=negmax[X][:, qt:qt + 1], scale=1.0,
                            accum_out=rowsum[X][:, qt:qt + 1],
                        )

                # --- recip + normalize (gpsimd) + transpose (alternating rings) ---
                rcp = {}
                for X in "AB":
                    r = st_pool.tile([128, QT], FP32, tag="rcp", name=f"rcp{X}")
                    nc.vector.reciprocal(r[:], rowsum[X][:])
                    rcp[X] = r
                # PT layout: [m_in, mt, qc, q_in(512)] so AV rhs slices are contiguous
                PT = {X: pt_pool.tile([128, MT, QC, 512], BF16, tag="pt", name=f"PT{X}") for X in "AB"}
                for X in "AB":
                    for qt in range(QT):
                        pn = pn_pool.tile([128, G], BF16, tag="pn")
                        nc.vector.tensor_scalar_mul(pn[:], P[X][:, qt], rcp[X][:, qt:qt + 1])
                        qc, qi = qt // 4, (qt % 4) * 128
                        tp_engines[tp_i % len(tp_engines)].dma_start_transpose(
                            out=PT[X][:, :, qc, qi:qi + 128], in_=pn[:])
                        tp_i += 1

                # --- AV (bf16 K=128, col-packed interleaved) + output ---
                o_sb = o_pool.tile([128, QC, 512], FP32, tag="o")
                for qc in range(QC):
                    pso = ps_vo.tile([128, 512], FP32, tag="vo", name=f"pso{qc}")
                    for mkt in range(MT):
                        st, sp = mkt == 0, mkt == MT - 1
                        nc.tensor.matmul(
                            pso[0:64, :], v_sb["A"][:, 64 * mkt:64 * (mkt + 1)],
                            PT["A"][:, mkt, qc, :],
                            start=st, stop=sp, skip_group_check=True,
                        )
                        nc.tensor.matmul(
                            pso[64:128, :], v_sb["B"][:, 64 * mkt:64 * (mkt + 1)],
                            PT["B"][:, mkt, qc, :],
                            start=st, stop=sp, skip_group_check=True,
                        )
                    nc.scalar.copy(o_sb[:, qc], pso[:])
                nc.gpsimd.dma_start(
                    out=out_ext[b, hA].rearrange("d (qc qi) -> d qc qi", qc=QC),
                    in_=o_sb[0:64],
                )
                nc.gpsimd.dma_start(
                    out=out_ext[b, hB].rearrange("d (qc qi) -> d qc qi", qc=QC),
                    in_=o_sb[64:128],
                )

    nc.compile()
    return nc


# ---------------------------------------------------------------------------
# Host-side wrapper: shard over batch across 8 cores, run SPMD, gather.
# ---------------------------------------------------------------------------
import numpy as np
import ml_dtypes

N_CORES = 8
_B_FULL, _NH, _G, _I, _D = 16, 8, 1024, 256, 64
_B_PER_CORE = _B_FULL // N_CORES

_cached_nc = None


def _get_nc():
    global _cached_nc
    if _cached_nc is None:
        _cached_nc = build_attention(B=_B_PER_CORE, NH=_NH, G=_G, I=_I, D=_D)
    return _cached_nc


def _make_in_maps(h, mask, W_Q, W_K, W_V):
    hT = np.ascontiguousarray(np.transpose(np.asarray(h, np.float32), (0, 2, 1)))
    wq = np.ascontiguousarray(np.asarray(W_Q, np.float32) / np.sqrt(np.float32(_D)))
    wk = np.ascontiguousarray(np.asarray(W_K, np.float32))
    wv = np.ascontiguousarray(np.asarray(W_V, np.float32))
    mask_i = np.ascontiguousarray(np.asarray(mask, np.int32))
    ident = np.eye(128).astype(ml_dtypes.bfloat16)
    return [
        {
            "hT": np.ascontiguousarray(hT[c * _B_PER_CORE:(c + 1) * _B_PER_CORE]),
            "mask": mask_i,
            "wq": wq,
            "wk": wk,
            "wv": wv,
            "ident": ident,
        }
        for c in range(N_CORES)
    ]


def kernel(h, mask, W_Q, W_K, W_V):
    """h [16,1024,256] f32, mask [1024,1024] i32, W_* [8,256,64] f32
    -> [16, 8, 1024, 64] f32"""
    from concourse.bass_utils import run_bass_kernel_spmd

    nc = _get_nc()
    in_maps = _make_in_maps(h, mask, W_Q, W_K, W_V)
    res = run_bass_kernel_spmd(nc, in_maps, core_ids=list(range(N_CORES)))
    outs = [np.asarray(res.results[c]["out"]).reshape(_B_PER_CORE, _NH, _D, _G)
            for c in range(N_CORES)]
    full = np.concatenate(outs, axis=0)              # [16, NH, D, G]
    return np.ascontiguousarray(full.transpose(0, 1, 3, 2)).astype(np.float32)
